# revision 1
# baseline (speedup 1.0000x reference)
"""GTE program-classification kernel for 8 Trainium2 NeuronCores.

Data-parallel over dst nodes: each core handles 1024 of the 8192 dst nodes.
Device kernel per core: embedding row gather (indirect DMA) -> 2-layer
post-norm transformer over the 8-message mailbox (bf16 GEMMs on PE,
attention on DVE/ACT) -> max-pool over messages -> linear classifier.
"""
import sys
if '/opt/trn_rl_repo' not in sys.path:
    sys.path.insert(0, '/opt/trn_rl_repo')

import numpy as np
import ml_dtypes

import concourse.bass as bass
import concourse.tile as tile
import concourse.mybir as mybir
from concourse.bass import ds
from concourse.bass_utils import run_bass_kernel_spmd

F32 = mybir.dt.float32
BF16 = mybir.dt.bfloat16
I32 = mybir.dt.int32
AF = mybir.ActivationFunctionType
OP = mybir.AluOpType
AX = mybir.AxisListType

P = 128
D = 512
H = 8
DH = 64
S = 8          # messages used per node (9th dropped by the reference)
NL = 2
V = 50000
NCLS = 104
DFF = 1024
NDST = 8192
NSRC = 40000
NCORES = 8
NLOC = NDST // NCORES      # 1024 dst nodes per core
NT = NLOC // P             # 8 node tiles per core
DC = D // P                # 4 d-chunks
FCH = DFF // P             # 8 dff-chunks
LN_EPS = 1e-5


def _split_multiwait_drains(nc):
    """walrus in this container accepts only one sync-wait per instruction;
    split any multi-wait Drain into a chain of single-wait drains."""
    for fn in nc.m.functions:
        for bb in fn.blocks:
            newlist = []
            for ins in bb.instructions:
                si = ins.sync_info
                if si is not None and si.on_wait and len(si.on_wait) > 1:
                    waits = list(si.on_wait)
                    for j, w in enumerate(waits[:-1]):
                        d = mybir.InstDrain(name=f'{ins.name}-sw{j}',
                                            engine=ins.engine)
                        d.sync_info = mybir.SyncInfo(on_wait=[w], on_update=[])
                        newlist.append(d)
                    si.on_wait = [waits[-1]]
                newlist.append(ins)
            bb.instructions[:] = newlist


def _layernorm(nc, sp, x, gam, bet):
    """In-place post-norm LN over d for each (node, s) token.
    x: [P, S, D] bf16 tile. gam/bet: replicated [P, D] f32 APs or None."""
    st = sp.tile([P, 6 * S], F32, tag="lnstat")
    sq = sp.tile([P, D], BF16, tag="lnsq")
    mean = st[:, 0:S]
    qs = st[:, S:2 * S]
    msq = st[:, 2 * S:3 * S]
    var = st[:, 3 * S:4 * S]
    rstd = st[:, 4 * S:5 * S]
    nmr = st[:, 5 * S:6 * S]
    for s in range(S):
        nc.vector.reduce_sum(mean[:, s:s + 1], x[:, s, :], axis=AX.X)
        nc.scalar.activation(sq[:], x[:, s, :], AF.Square,
                             accum_out=qs[:, s:s + 1])
    nc.vector.tensor_scalar_mul(mean[:], mean[:], 1.0 / D)
    nc.vector.tensor_tensor(out=msq[:], in0=mean[:], in1=mean[:], op=OP.mult)
    nc.vector.scalar_tensor_tensor(out=var[:], in0=qs[:], scalar=1.0 / D,
                                   in1=msq[:], op0=OP.mult, op1=OP.subtract)
    nc.vector.tensor_scalar_add(var[:], var[:], LN_EPS)
    nc.scalar.activation(rstd[:], var[:], AF.Sqrt)
    nc.vector.reciprocal(rstd[:], rstd[:])
    nc.vector.scalar_tensor_tensor(out=nmr[:], in0=mean[:], scalar=-1.0,
                                   in1=rstd[:], op0=OP.mult, op1=OP.mult)
    for s in range(S):
        nc.scalar.activation(x[:, s, :], x[:, s, :], AF.Identity,
                             bias=nmr[:, s:s + 1], scale=rstd[:, s:s + 1])
    if gam is not None:
        for s in range(S):
            nc.vector.tensor_tensor(out=x[:, s, :], in0=x[:, s, :], in1=gam,
                                    op=OP.mult)
    if bet is not None:
        for s in range(S):
            nc.vector.tensor_tensor(out=x[:, s, :], in0=x[:, s, :], in1=bet,
                                    op=OP.add)


def build(flags):
    nc = bass.Bass()

    emb_d = nc.dram_tensor("embb", [V, D], BF16, kind="ExternalInput")
    idx_d = nc.dram_tensor("tid2", [NLOC, S], I32, kind="ExternalInput")
    wq_d = nc.dram_tensor("wqkvT", [NL, D, 3 * D], BF16, kind="ExternalInput")
    wo_d = nc.dram_tensor("woT", [NL, D, D], BF16, kind="ExternalInput")
    w1_d = nc.dram_tensor("w1T", [NL, D, DFF], BF16, kind="ExternalInput")
    w2_d = nc.dram_tensor("w2T", [NL, DFF, D], BF16, kind="ExternalInput")
    wf_d = nc.dram_tensor("wfcT", [D, NCLS], BF16, kind="ExternalInput")
    out_d = nc.dram_tensor("logits", [NLOC, NCLS], F32, kind="ExternalOutput")

    need_vec = {}
    if flags['bqkv']:
        need_vec['bqkv'] = [NL, 3 * D]
    if flags['bo']:
        need_vec['bo'] = [NL, D]
    # b1 is consumed in h^T (dff-on-partition) space: host passes it
    # pre-transposed as [P, NL*FCH] so partition p, col l*FCH+m holds
    # b1[l, m*P + p]; loaded by plain DMA below (not replicated).
    if flags['b2']:
        need_vec['b2'] = [NL, D]
    if flags['bfc']:
        need_vec['bfc'] = [1, NCLS]
    if flags['ln_g']:
        need_vec['ln1_g'] = [NL, D]
        need_vec['ln2_g'] = [NL, D]
    if flags['ln_b']:
        need_vec['ln1_b'] = [NL, D]
        need_vec['ln2_b'] = [NL, D]
    vec_d = {k: nc.dram_tensor(k, shp, F32, kind="ExternalInput")
             for k, shp in need_vec.items()}
    b1t_d = (nc.dram_tensor("b1t", [P, NL * FCH], F32, kind="ExternalInput")
             if flags['b1'] else None)

    with tile.TileContext(nc) as tc:
        with tc.tile_pool(name="wpool", bufs=1) as wp, \
             tc.tile_pool(name="big1", bufs=1) as b1p, \
             tc.tile_pool(name="dbl", bufs=2) as dbl, \
             tc.tile_pool(name="sp", bufs=2) as sp, \
             tc.tile_pool(name="psA", bufs=2, space="PSUM") as psA, \
             tc.tile_pool(name="psB", bufs=2, space="PSUM") as psB:

            # ---- resident weights (bf16) ----
            wq_sb, wo_sb, w1_sb, w2_sb = [], [], [], []
            for l in range(NL):
                t = wp.tile([P, DC, 3 * D], BF16, tag=f"wq{l}")
                for c in range(DC):
                    nc.sync.dma_start(t[:, c, :], wq_d[l, c * P:(c + 1) * P, :])
                wq_sb.append(t)
                t = wp.tile([P, DC, D], BF16, tag=f"wo{l}")
                for c in range(DC):
                    nc.sync.dma_start(t[:, c, :], wo_d[l, c * P:(c + 1) * P, :])
                wo_sb.append(t)
                t = wp.tile([P, DC, DFF], BF16, tag=f"w1{l}")
                for c in range(DC):
                    nc.sync.dma_start(t[:, c, :], w1_d[l, c * P:(c + 1) * P, :])
                w1_sb.append(t)
                t = wp.tile([P, FCH, D], BF16, tag=f"w2{l}")
                for c in range(FCH):
                    nc.sync.dma_start(t[:, c, :], w2_d[l, c * P:(c + 1) * P, :])
                w2_sb.append(t)
            wf_sb = wp.tile([P, DC, NCLS], BF16, tag="wf")
            for c in range(DC):
                nc.sync.dma_start(wf_sb[:, c, :], wf_d[c * P:(c + 1) * P, :])

            vec_sb = {}
            for k, shp in need_vec.items():
                n = shp[0] * shp[1]
                t0 = wp.tile([1, n], F32, tag=f"{k}_row")
                nc.sync.dma_start(t0[:, :],
                                  vec_d[k][:].rearrange("a b -> 1 (a b)"))
                tb = wp.tile([P, n], F32, tag=f"{k}_rep")
                nc.gpsimd.partition_broadcast(tb[:], t0[:])
                vec_sb[k] = tb

            b1t_sb = None
            if flags['b1']:
                b1t_sb = wp.tile([P, NL * FCH], F32, tag="b1t")
                nc.sync.dma_start(b1t_sb[:], b1t_d[:])

            def vsl(k, l, n):
                return vec_sb[k][:, l * n:(l + 1) * n]

            def body(i):
                idx_sb = dbl.tile([P, S], I32, tag="idx")
                nc.sync.dma_start(idx_sb[:], idx_d[ds(i * P, P), :])

                x = dbl.tile([P, S, D], BF16, tag="x")
                for s in range(S):
                    nc.gpsimd.indirect_dma_start(
                        out=x[:, s, :], out_offset=None, in_=emb_d[:],
                        in_offset=bass.IndirectOffsetOnAxis(
                            ap=idx_sb[:, s:s + 1], axis=0))

                for l in range(NL):
                    # x^T chunks: [128d, 1024tok], tok = s*128 + node
                    xT = b1p.tile([P, DC, S * P], BF16, tag="xT")
                    for s in range(S):
                        for c in range(DC):
                            nc.sync.dma_start_transpose(
                                xT[:, c, s * P:(s + 1) * P],
                                x[:, s, c * P:(c + 1) * P])

                    # QKV (q pre-scaled by 1/8)
                    qkv = b1p.tile([P, S, 3 * D], BF16, tag="qkv")
                    for s in range(S):
                        pq = psA.tile([P, 3 * D], F32, tag="pqkv")
                        for c in range(DC):
                            lhsT = xT[:, c, s * P:(s + 1) * P]
                            for nb in range(3):
                                nc.tensor.matmul(
                                    pq[:, nb * D:(nb + 1) * D], lhsT,
                                    wq_sb[l][:, c, nb * D:(nb + 1) * D],
                                    start=(c == 0), stop=(c == DC - 1))
                        if flags['bqkv']:
                            nc.vector.tensor_add(pq[:], pq[:],
                                                 vsl('bqkv', l, 3 * D))
                        nc.scalar.activation(qkv[:, s, 0:D], pq[:, 0:D],
                                             AF.Copy, scale=0.125)
                        nc.scalar.copy(qkv[:, s, D:3 * D], pq[:, D:3 * D])

                    # ---- attention ----
                    # scores laid [P, s, h, t]
                    scores = dbl.tile([P, S, H, S], F32, tag="scores")
                    for s in range(S):
                        qk = b1p.tile([P, S, D], BF16, tag="qk")
                        nc.vector.tensor_tensor(
                            out=qk[:],
                            in0=qkv[:, :, D:2 * D],
                            in1=qkv[:, s, 0:D].unsqueeze(1)
                                .broadcast_to([P, S, D]),
                            op=OP.mult)
                        # segmented d-reduce: in-segment halving tree (bf16,
                        # 2x DVE mode) then a short 1x reduce over 8
                        qk4 = qk[:].rearrange("p t (h e) -> p t h e", h=H)
                        for w in (32, 16, 8):
                            nc.vector.tensor_add(qk4[:, :, :, 0:w],
                                                 qk4[:, :, :, 0:w],
                                                 qk4[:, :, :, w:2 * w])
                        nc.vector.reduce_sum(
                            scores[:, s, :, :].transpose([0, 2, 1]),
                            qk4[:, :, :, 0:8],
                            axis=AX.X)
                    # softmax over t (post-LN scores are O(1); skip max-sub)
                    pexp = dbl.tile([P, S, H, S], F32, tag="pexp")
                    nc.scalar.activation(
                        pexp[:].rearrange("p s h t -> p (s h t)"),
                        scores[:].rearrange("p s h t -> p (s h t)"), AF.Exp)
                    den = sp.tile([P, S * H], F32, tag="den")
                    nc.vector.reduce_sum(
                        den[:].rearrange("p (s h) -> p s h", s=S),
                        pexp[:], axis=AX.X)
                    nc.vector.reciprocal(den[:], den[:])
                    pn = dbl.tile([P, S, H, S], BF16, tag="pn")
                    nc.vector.tensor_tensor(
                        out=pn[:], in0=pexp[:],
                        in1=den[:].rearrange("p (s h) -> p s h", s=S)
                            .unsqueeze(3).broadcast_to([P, S, H, S]),
                        op=OP.mult)

                    # AV: a[:, s, (h d)] = sum_t pn[s,h,t] * v[t, (h d)]
                    a = b1p.tile([P, S, D], BF16, tag="attn")
                    for s in range(S):
                        # expand pn over d on ACT so the DVE multiply gets
                        # dense step-1 operands (2x bf16 mode)
                        pnx = b1p.tile([P, S, H, DH], BF16, tag="pnx")
                        nc.scalar.copy(
                            pnx[:],
                            pn[:, s, :, :].transpose([0, 2, 1])
                            .unsqueeze(3).broadcast_to([P, S, H, DH]))
                        av = b1p.tile([P, S, H, DH], BF16, tag="av")
                        nc.vector.tensor_tensor(
                            out=av[:],
                            in0=qkv[:, :, 2 * D:3 * D]
                                .rearrange("p t (h e) -> p t h e", h=H),
                            in1=pnx[:],
                            op=OP.mult)
                        avf = av[:].rearrange("p t h e -> p t (h e)")
                        nc.vector.tensor_add(avf[:, 0:4, :], avf[:, 0:4, :],
                                             avf[:, 4:8, :])
                        nc.vector.tensor_add(avf[:, 0:2, :], avf[:, 0:2, :],
                                             avf[:, 2:4, :])
                        nc.vector.tensor_add(a[:, s, :], avf[:, 0, :],
                                             avf[:, 1, :])

                    # a^T; Wo; residual into x
                    aT = b1p.tile([P, DC, S * P], BF16, tag="aT")
                    for s in range(S):
                        for c in range(DC):
                            nc.sync.dma_start_transpose(
                                aT[:, c, s * P:(s + 1) * P],
                                a[:, s, c * P:(c + 1) * P])
                    for s in range(S):
                        po = psB.tile([P, D], F32, tag="mm")
                        for c in range(DC):
                            nc.tensor.matmul(po[:],
                                             aT[:, c, s * P:(s + 1) * P],
                                             wo_sb[l][:, c, :],
                                             start=(c == 0), stop=(c == DC - 1))
                        if flags['bo']:
                            nc.vector.tensor_add(po[:], po[:], vsl('bo', l, D))
                        ob = sp.tile([P, D], BF16, tag="ob")
                        nc.scalar.copy(ob[:], po[:])
                        nc.vector.tensor_add(x[:, s, :], x[:, s, :], ob[:])

                    _layernorm(nc, sp, x,
                               vsl('ln1_g', l, D) if flags['ln_g'] else None,
                               vsl('ln1_b', l, D) if flags['ln_b'] else None)

                    # x1^T for FFN1
                    x1T = b1p.tile([P, DC, S * P], BF16, tag="x1T")
                    for s in range(S):
                        for c in range(DC):
                            nc.sync.dma_start_transpose(
                                x1T[:, c, s * P:(s + 1) * P],
                                x[:, s, c * P:(c + 1) * P])

                    # FFN1 -> h^T [dff-part, tok], fused relu (+b1)
                    hT = b1p.tile([P, FCH, S * P], BF16, tag="hT")
                    for m in range(FCH):
                        for hf in range(2):
                            ph = psB.tile([P, D], F32, tag="mm")
                            for c in range(DC):
                                nc.tensor.matmul(
                                    ph[:],
                                    w1_sb[l][:, c, m * P:(m + 1) * P],
                                    x1T[:, c, hf * D:(hf + 1) * D],
                                    start=(c == 0), stop=(c == DC - 1))
                            if flags['b1']:
                                nc.scalar.activation(
                                    hT[:, m, hf * D:(hf + 1) * D], ph[:],
                                    AF.Relu,
                                    bias=b1t_sb[:, l * FCH + m:l * FCH + m + 1])
                            else:
                                nc.scalar.activation(
                                    hT[:, m, hf * D:(hf + 1) * D], ph[:],
                                    AF.Relu)

                    # FFN2 + residual + LN2
                    for s in range(S):
                        pf = psB.tile([P, D], F32, tag="mm")
                        for k in range(FCH):
                            nc.tensor.matmul(pf[:],
                                             hT[:, k, s * P:(s + 1) * P],
                                             w2_sb[l][:, k, :],
                                             start=(k == 0),
                                             stop=(k == FCH - 1))
                        if flags['b2']:
                            nc.vector.tensor_add(pf[:], pf[:], vsl('b2', l, D))
                        fb = sp.tile([P, D], BF16, tag="fb")
                        nc.scalar.copy(fb[:], pf[:])
                        nc.vector.tensor_add(x[:, s, :], x[:, s, :], fb[:])
                    _layernorm(nc, sp, x,
                               vsl('ln2_g', l, D) if flags['ln_g'] else None,
                               vsl('ln2_b', l, D) if flags['ln_b'] else None)

                # ---- max-pool over s + classifier ----
                nc.vector.tensor_tensor(out=x[:, 0:4, :], in0=x[:, 0:4, :],
                                        in1=x[:, 4:8, :], op=OP.max)
                nc.vector.tensor_tensor(out=x[:, 0:2, :], in0=x[:, 0:2, :],
                                        in1=x[:, 2:4, :], op=OP.max)
                rst = sp.tile([P, D], BF16, tag="rst")
                nc.vector.tensor_tensor(out=rst[:], in0=x[:, 0, :],
                                        in1=x[:, 1, :], op=OP.max)
                rT = sp.tile([P, DC, P], BF16, tag="rT")
                for c in range(DC):
                    nc.sync.dma_start_transpose(rT[:, c, :],
                                                rst[:, c * P:(c + 1) * P])
                pc = psB.tile([P, D], F32, tag="mm")
                for c in range(DC):
                    nc.tensor.matmul(pc[:, 0:NCLS], rT[:, c, :], wf_sb[:, c, :],
                                     start=(c == 0), stop=(c == DC - 1))
                if flags['bfc']:
                    nc.vector.tensor_add(pc[:, 0:NCLS], pc[:, 0:NCLS],
                                         vec_sb['bfc'][:, :])
                lg = sp.tile([P, NCLS], F32, tag="lg")
                nc.vector.tensor_copy(lg[:], pc[:, 0:NCLS])
                nc.sync.dma_start(out_d[ds(i * P, P), :], lg[:])

            # For_i + in-loop SWDGE (indirect gather) emits InstIncSwdgeSem,
            # which this container's walrus cannot encode -> fully unroll.
            for i in range(NT):
                body(i)

    _split_multiwait_drains(nc)
    return nc


OPT_KEYS = ('bqkv', 'bo', 'b1', 'b2', 'bfc', 'ln_g', 'ln_b')
_cache = {}


def _get_nc(flags):
    key = tuple(flags[k] for k in OPT_KEYS)
    if key not in _cache:
        _cache[key] = build(flags)
    return _cache[key]


def kernel(**inputs):
    token_ids = np.asarray(inputs['token_ids'])
    edge_src = np.asarray(inputs['edge_src'])
    emb = np.asarray(inputs['emb'], dtype=np.float32)
    Wqkv = np.asarray(inputs['Wqkv'], dtype=np.float32)
    bqkv = np.asarray(inputs['bqkv'], dtype=np.float32)
    Wo = np.asarray(inputs['Wo'], dtype=np.float32)
    bo = np.asarray(inputs['bo'], dtype=np.float32)
    W1 = np.asarray(inputs['W1'], dtype=np.float32)
    b1 = np.asarray(inputs['b1'], dtype=np.float32)
    W2 = np.asarray(inputs['W2'], dtype=np.float32)
    b2 = np.asarray(inputs['b2'], dtype=np.float32)
    ln1_g = np.asarray(inputs['ln1_g'], dtype=np.float32)
    ln1_b = np.asarray(inputs['ln1_b'], dtype=np.float32)
    ln2_g = np.asarray(inputs['ln2_g'], dtype=np.float32)
    ln2_b = np.asarray(inputs['ln2_b'], dtype=np.float32)
    Wfc = np.asarray(inputs['Wfc'], dtype=np.float32)
    bfc = np.asarray(inputs['bfc'], dtype=np.float32)

    flags = {
        'bqkv': bool(np.any(bqkv)), 'bo': bool(np.any(bo)),
        'b1': bool(np.any(b1)), 'b2': bool(np.any(b2)),
        'bfc': bool(np.any(bfc)),
        'ln_g': bool(np.any(ln1_g != 1.0) or np.any(ln2_g != 1.0)),
        'ln_b': bool(np.any(ln1_b) or np.any(ln2_b)),
    }
    nc = _get_nc(flags)

    bf = ml_dtypes.bfloat16
    tid2 = token_ids[edge_src[:, :S]].astype(np.int32)     # [NDST, S]
    embb = emb.astype(bf)
    wqkvT = np.ascontiguousarray(Wqkv.transpose(0, 2, 1)).astype(bf)
    woT = np.ascontiguousarray(Wo.transpose(0, 2, 1)).astype(bf)
    w1T = np.ascontiguousarray(W1.transpose(0, 2, 1)).astype(bf)
    w2T = np.ascontiguousarray(W2.transpose(0, 2, 1)).astype(bf)
    wfcT = np.ascontiguousarray(Wfc.T).astype(bf)

    common = {
        'embb': embb, 'wqkvT': wqkvT, 'woT': woT, 'w1T': w1T, 'w2T': w2T,
        'wfcT': wfcT,
    }
    if flags['bqkv']:
        common['bqkv'] = bqkv
    if flags['bo']:
        common['bo'] = bo
    if flags['b1']:
        # [P, NL*FCH]: partition p, col l*FCH+m = b1[l, m*P+p]
        common['b1t'] = np.ascontiguousarray(
            b1.reshape(NL, FCH, P).transpose(2, 0, 1).reshape(P, NL * FCH))
    if flags['b2']:
        common['b2'] = b2
    if flags['bfc']:
        common['bfc'] = bfc.reshape(1, NCLS)
    if flags['ln_g']:
        common['ln1_g'] = ln1_g
        common['ln2_g'] = ln2_g
    if flags['ln_b']:
        common['ln1_b'] = ln1_b
        common['ln2_b'] = ln2_b

    in_maps = []
    for c in range(NCORES):
        m = dict(common)
        m['tid2'] = np.ascontiguousarray(tid2[c * NLOC:(c + 1) * NLOC])
        in_maps.append(m)

    res = run_bass_kernel_spmd(nc, in_maps, core_ids=list(range(NCORES)))
    out = np.concatenate([res.results[c]['logits'] for c in range(NCORES)],
                         axis=0)
    return out.astype(np.float32)


if __name__ == '__main__':
    import time
    sys.path.insert(0, '/root/problem')
    import reference
    inp = {k: np.asarray(v) for k, v in reference.setup_inputs().items()}
    t0 = time.time()
    got = kernel(**inp)
    print(f"kernel ran in {time.time()-t0:.1f}s")
    exp = np.asarray(reference.reference(**reference.setup_inputs()))
    err = np.abs(got - exp).max()
    rel = err / np.abs(exp).max()
    print(f"absmax err {err:.3e}  rel {rel:.3e}")



# revision 17
# speedup vs baseline: 8.9539x; 8.9539x over previous
"""GTE program-classification kernel for 8 Trainium2 NeuronCores.

Data-parallel over dst nodes: each core handles 1024 of the 8192 dst nodes.
Device kernel per core: embedding row gather (indirect DMA) -> 2-layer
post-norm transformer over the 8-message mailbox (bf16 GEMMs on PE,
attention on DVE/Pool, softmax exp on ACT) -> max-pool -> classifier.

v2 layout notes:
- Head dims are stored h-innermost ((e, h) column order, h fastest) via a
  host-side permutation of the QKV output columns and Wo input rows. Every
  attention elementwise op then has a packed (step-1, >=2 elem) last dim on
  all operands, so the DVE runs them in 2x bf16 mode, and broadcast
  expansion tensors are never materialized on ACT.
- LN stats come from ACT Square+accum (sumsq) and a DVE 4x tensor_scalar
  accumulate (sum); rstd = exp(-0.5*ln(var+eps)) keeps the ACT engine on a
  single activation table (natural_log_exp_and_others) for the whole
  kernel -- no 1.28us table reloads.
- The scores/AV work for a tunable subset of the 8 mailbox slots runs on
  the otherwise-idle Pool (gpsimd) engine.
"""
import sys
if '/opt/trn_rl_repo' not in sys.path:
    sys.path.insert(0, '/opt/trn_rl_repo')

import numpy as np
import ml_dtypes

import concourse.bass as bass
import concourse.tile as tile
import concourse.mybir as mybir
from concourse.bass import ds
from concourse.bass_utils import run_bass_kernel_spmd

F32 = mybir.dt.float32
BF16 = mybir.dt.bfloat16
I32 = mybir.dt.int32
AF = mybir.ActivationFunctionType
OP = mybir.AluOpType
AX = mybir.AxisListType

P = 128
D = 512
H = 8
DH = 64
S = 8          # messages used per node (9th dropped by the reference)
NL = 2
V = 50000
NCLS = 104
DFF = 1024
NDST = 8192
NSRC = 40000
NCORES = 8
NLOC = NDST // NCORES      # 1024 dst nodes per core
NT = NLOC // P             # 8 node tiles per core
DC = D // P                # 4 d-chunks
FCH = DFF // P             # 8 dff-chunks
LN_EPS = 1e-5

# engine assignment knobs: which s-slots of scores / AV run on Pool
SC_POOL_S = (7,)
AV_POOL_S = (5, 6, 7)
SKEW = 3       # pipeline skew (stages) between consecutive node tiles


def _split_multiwait_drains(nc):
    """walrus in this container accepts only one sync-wait per instruction;
    split any multi-wait Drain into a chain of single-wait drains."""
    for fn in nc.m.functions:
        for bb in fn.blocks:
            newlist = []
            for ins in bb.instructions:
                si = ins.sync_info
                if si is not None and si.on_wait and len(si.on_wait) > 1:
                    waits = list(si.on_wait)
                    for j, w in enumerate(waits[:-1]):
                        d = mybir.InstDrain(name=f'{ins.name}-sw{j}',
                                            engine=ins.engine)
                        d.sync_info = mybir.SyncInfo(on_wait=[w], on_update=[])
                        newlist.append(d)
                    si.on_wait = [waits[-1]]
                newlist.append(ins)
            bb.instructions[:] = newlist


def _layernorm(nc, sp2, sp1, x, gam, bet, eps_ap):
    """In-place post-norm LN over d for each (node, s) token.
    x: [P, S, D] bf16 tile. gam/bet: replicated [P, D] f32 APs or None."""
    st = sp2.tile([P, 6 * S], F32, tag="lnstat")
    # scratch outputs: reuse the (dead-at-LN-time) ob/fb buffers
    sq = sp1.tile([P, D], BF16, tag="ob")
    tr = sp1.tile([P, D], BF16, tag="fb")
    sums = st[:, 0:S]
    qs = st[:, S:2 * S]
    mean = st[:, 2 * S:3 * S]
    msq = st[:, 3 * S:4 * S]
    var = st[:, 4 * S:5 * S]
    rstd = st[:, 5 * S:6 * S]
    for s in range(S):
        nc.scalar.activation(sq[:], x[:, s, :], AF.Square,
                             accum_out=qs[:, s:s + 1])
        nc.vector.tensor_scalar(out=tr[:], in0=x[:, s, :], scalar1=1.0,
                                scalar2=None, op0=OP.mult, op1=OP.add,
                                accum_out=sums[:, s:s + 1])
    nc.vector.tensor_scalar_mul(mean[:], sums[:], 1.0 / D)
    nc.vector.tensor_tensor(out=msq[:], in0=mean[:], in1=mean[:], op=OP.mult)
    nc.vector.scalar_tensor_tensor(out=var[:], in0=qs[:], scalar=1.0 / D,
                                   in1=msq[:], op0=OP.mult, op1=OP.subtract)
    # rstd = (var+eps)^-1/2 = exp(-0.5*ln(var+eps)); Ln/Exp share the ACT
    # table with Copy/Square/Relu so no table reload is triggered.
    nc.scalar.activation(var[:], var[:], AF.Ln, bias=eps_ap)
    nc.scalar.activation(rstd[:], var[:], AF.Exp, scale=-0.5)
    for s in range(S):
        nc.vector.tensor_scalar(out=x[:, s, :], in0=x[:, s, :],
                                scalar1=mean[:, s:s + 1],
                                scalar2=rstd[:, s:s + 1],
                                op0=OP.subtract, op1=OP.mult)
    if gam is not None:
        for s in range(S):
            nc.vector.tensor_tensor(out=x[:, s, :], in0=x[:, s, :], in1=gam,
                                    op=OP.mult)
    if bet is not None:
        for s in range(S):
            nc.vector.tensor_tensor(out=x[:, s, :], in0=x[:, s, :], in1=bet,
                                    op=OP.add)


def build(flags):
    nc = bass.Bass()

    emb_d = nc.dram_tensor("embb", [V, D], BF16, kind="ExternalInput")
    idx_d = nc.dram_tensor("tid2", [NLOC, S], I32, kind="ExternalInput")
    wq_d = nc.dram_tensor("wqkvT", [NL, D, 3 * D], BF16, kind="ExternalInput")
    wo_d = nc.dram_tensor("woT", [NL, D, D], BF16, kind="ExternalInput")
    w1_d = nc.dram_tensor("w1T", [NL, D, DFF], BF16, kind="ExternalInput")
    w2_d = nc.dram_tensor("w2T", [NL, DFF, D], BF16, kind="ExternalInput")
    wf_d = nc.dram_tensor("wfcT", [D, NCLS], BF16, kind="ExternalInput")
    out_d = nc.dram_tensor("logits", [NLOC, NCLS], F32, kind="ExternalOutput")

    need_vec = {}
    if flags['bqkv']:
        need_vec['bqkv'] = [NL, 3 * D]
    if flags['bo']:
        need_vec['bo'] = [NL, D]
    # b1 is consumed in h^T (dff-on-partition) space: host passes it
    # pre-transposed as [P, NL*FCH] (partition p, col l*FCH+m = b1[l, m*P+p])
    if flags['b2']:
        need_vec['b2'] = [NL, D]
    if flags['bfc']:
        need_vec['bfc'] = [1, NCLS]
    if flags['ln_g']:
        need_vec['ln1_g'] = [NL, D]
        need_vec['ln2_g'] = [NL, D]
    if flags['ln_b']:
        need_vec['ln1_b'] = [NL, D]
        need_vec['ln2_b'] = [NL, D]
    vec_d = {k: nc.dram_tensor(k, shp, F32, kind="ExternalInput")
             for k, shp in need_vec.items()}
    b1t_d = (nc.dram_tensor("b1t", [P, NL * FCH], F32, kind="ExternalInput")
             if flags['b1'] else None)

    with tile.TileContext(nc) as tc:
        with tc.tile_pool(name="wpool", bufs=1) as wp, \
             tc.tile_pool(name="big1", bufs=1) as b1p, \
             tc.tile_pool(name="qkvp", bufs=2) as qkvp, \
             tc.tile_pool(name="dbl", bufs=2) as dbl, \
             tc.tile_pool(name="xp", bufs=3) as xp, \
             tc.tile_pool(name="prod", bufs=1) as prod, \
             tc.tile_pool(name="sp", bufs=1) as sp, \
             tc.tile_pool(name="sp1", bufs=1) as sp1, \
             tc.tile_pool(name="psA", bufs=2, space="PSUM") as psA, \
             tc.tile_pool(name="psB", bufs=2, space="PSUM") as psB:

            # ---- resident weights (bf16) ----
            wq_sb, wo_sb, w1_sb, w2_sb = [], [], [], []
            for l in range(NL):
                t = wp.tile([P, DC, 3 * D], BF16, tag=f"wq{l}")
                for c in range(DC):
                    nc.sync.dma_start(t[:, c, :], wq_d[l, c * P:(c + 1) * P, :])
                wq_sb.append(t)
                t = wp.tile([P, DC, D], BF16, tag=f"wo{l}")
                for c in range(DC):
                    nc.sync.dma_start(t[:, c, :], wo_d[l, c * P:(c + 1) * P, :])
                wo_sb.append(t)
                t = wp.tile([P, DC, DFF], BF16, tag=f"w1{l}")
                for c in range(DC):
                    nc.sync.dma_start(t[:, c, :], w1_d[l, c * P:(c + 1) * P, :])
                w1_sb.append(t)
                t = wp.tile([P, FCH, D], BF16, tag=f"w2{l}")
                for c in range(FCH):
                    nc.sync.dma_start(t[:, c, :], w2_d[l, c * P:(c + 1) * P, :])
                w2_sb.append(t)
            wf_sb = wp.tile([P, DC, NCLS], BF16, tag="wf")
            for c in range(DC):
                nc.sync.dma_start(wf_sb[:, c, :], wf_d[c * P:(c + 1) * P, :])

            vec_sb = {}
            for k, shp in need_vec.items():
                n = shp[0] * shp[1]
                t0 = wp.tile([1, n], F32, tag=f"{k}_row")
                nc.sync.dma_start(t0[:, :],
                                  vec_d[k][:].rearrange("a b -> 1 (a b)"))
                tb = wp.tile([P, n], F32, tag=f"{k}_rep")
                nc.gpsimd.partition_broadcast(tb[:], t0[:])
                vec_sb[k] = tb

            b1t_sb = None
            if flags['b1']:
                b1t_sb = wp.tile([P, NL * FCH], F32, tag="b1t")
                nc.sync.dma_start(b1t_sb[:], b1t_d[:])

            eps_sb = wp.tile([P, 1], F32, tag="eps")
            nc.vector.memset(eps_sb[:], LN_EPS)

            def vsl(k, l, n):
                return vec_sb[k][:, l * n:(l + 1) * n]

            # ---- per-tile work, split into software-pipeline stages ----
            # stage list per tile: [gather, (qkv, attn, wo+ln1, ffn+ln2) x2,
            # classifier]; tiles are emitted with a skew of SKEW stages so
            # each in-order engine always has independent work queued.
            state = [dict() for _ in range(NT)]

            def stage_gather(i):
                st = state[i]
                idx_sb = dbl.tile([P, S], I32, tag="idx")
                nc.sync.dma_start(idx_sb[:], idx_d[ds(i * P, P), :])
                x = st['x'] = xp.tile([P, S, D], BF16, tag="x", name="x")
                for s in range(S):
                    nc.gpsimd.indirect_dma_start(
                        out=x[:, s, :], out_offset=None, in_=emb_d[:],
                        in_offset=bass.IndirectOffsetOnAxis(
                            ap=idx_sb[:, s:s + 1], axis=0))

            def stage_qkv(i, l):
                st = state[i]
                x = st['x']
                # x^T chunks: [128d, 1024tok], tok = s*128 + node
                xT = b1p.tile([P, DC, S * P], BF16, tag="xT")
                for s in range(S):
                    for c in range(DC):
                        nc.sync.dma_start_transpose(
                            xT[:, c, s * P:(s + 1) * P],
                            x[:, s, c * P:(c + 1) * P])

                # QKV; q-scale (1/8) and (e,h) column order are folded
                # into the weights host-side.
                qkv = st['qkv'] = qkvp.tile([P, S, 3 * D], BF16, tag="qkv", name="qkv")
                for s in range(S):
                        pq = psA.tile([P, 3 * D], F32, tag="pqkv")
                        for c in range(DC):
                            lhsT = xT[:, c, s * P:(s + 1) * P]
                            for nb in range(3):
                                nc.tensor.matmul(
                                    pq[:, nb * D:(nb + 1) * D], lhsT,
                                    wq_sb[l][:, c, nb * D:(nb + 1) * D],
                                    start=(c == 0), stop=(c == DC - 1))
                        if flags['bqkv']:
                            nc.vector.tensor_add(pq[:], pq[:],
                                                 vsl('bqkv', l, 3 * D))
                        nc.scalar.copy(qkv[:, s, :], pq[:])

            def stage_attn(i, l):
                st = state[i]
                qkv = st['qkv']
                # scores[p, s, t, h] = sum_e q[p,s,(e,h)] k[p,t,(e,h)]
                scores = sp.tile([P, S, S, H], BF16, tag="scores")
                for s in range(S):
                    eng = nc.gpsimd if s in SC_POOL_S else nc.vector
                    tg = "prod_p" if s in SC_POOL_S else "prod_d"
                    qk = prod.tile([P, S, D], BF16, tag=tg)
                    eng.tensor_tensor(
                        out=qk[:],
                        in0=qkv[:, :, D:2 * D],
                        in1=qkv[:, s, 0:D].unsqueeze(1)
                            .broadcast_to([P, S, D]),
                        op=OP.mult)
                    qk4 = qk[:].rearrange("p t (e h) -> p t e h", h=H)
                    for w in (32, 16, 8, 4, 2):
                        eng.tensor_add(qk4[:, :, 0:w, :],
                                       qk4[:, :, 0:w, :],
                                       qk4[:, :, w:2 * w, :])
                    eng.tensor_add(scores[:, s, :, :],
                                   qk4[:, :, 0, :], qk4[:, :, 1, :])

                # softmax over t (post-LN scores are O(1); skip max-sub)
                pexp = sp.tile([P, S, S, H], BF16, tag="pexp")
                nc.scalar.activation(
                    pexp[:].rearrange("p s t h -> p (s t h)"),
                    scores[:].rearrange("p s t h -> p (s t h)"), AF.Exp)
                den4 = sp.tile([P, S, 4, H], BF16, tag="den4")
                den1 = sp.tile([P, S, 1, H], F32, tag="den1")
                dr = sp.tile([P, S, 1, H], BF16, tag="dr")
                nc.vector.tensor_add(den4[:], pexp[:, :, 0:4, :],
                                     pexp[:, :, 4:8, :])
                nc.vector.tensor_add(den4[:, :, 0:2, :],
                                     den4[:, :, 0:2, :],
                                     den4[:, :, 2:4, :])
                nc.vector.tensor_add(den1[:], den4[:, :, 0:1, :],
                                     den4[:, :, 1:2, :])
                with nc.allow_low_precision(
                        reason="softmax denom reciprocal in bf16; "
                               "|err|~0.4% vs 2e-2 tolerance"):
                    nc.vector.reciprocal(dr[:], den1[:])
                pn = scores   # overwrite scores (already consumed by exp)
                nc.vector.tensor_tensor(
                    out=pn[:], in0=pexp[:],
                    in1=dr[:].broadcast_to([P, S, S, H]), op=OP.mult)

                # AV: a[p, s, (e,h)] = sum_t pn[p,s,t,h] v[p,t,(e,h)]
                a = st['a'] = b1p.tile([P, S, D], BF16, tag="attn", name="attn")
                v4 = qkv[:, :, 2 * D:3 * D].rearrange(
                    "p t (e h) -> p t e h", h=H)
                for s in range(S):
                    eng = nc.gpsimd if s in AV_POOL_S else nc.vector
                    tg = "prod_p" if s in AV_POOL_S else "prod_d"
                    av = prod.tile([P, S, D], BF16, tag=tg)
                    av4 = av[:].rearrange("p t (e h) -> p t e h", h=H)
                    eng.tensor_tensor(
                        out=av4[:],
                        in0=v4,
                        in1=pn[:, s, :, :].unsqueeze(2)
                            .broadcast_to([P, S, DH, H]),
                        op=OP.mult)
                    eng.tensor_add(av[:, 0:4, :], av[:, 0:4, :],
                                   av[:, 4:8, :])
                    eng.tensor_add(av[:, 0:2, :], av[:, 0:2, :],
                                   av[:, 2:4, :])
                    eng.tensor_add(a[:, s, :], av[:, 0, :], av[:, 1, :])

                # a^T for the Wo matmul
                aT = st['aT'] = b1p.tile([P, DC, S * P], BF16, tag="aT", name="aT")
                for s in range(S):
                    for c in range(DC):
                        nc.sync.dma_start_transpose(
                            aT[:, c, s * P:(s + 1) * P],
                            a[:, s, c * P:(c + 1) * P])

            def stage_wo_ln1(i, l):
                st = state[i]
                x, aT = st['x'], st['aT']
                for s in range(S):
                    po = psB.tile([P, D], F32, tag="mm")
                    for c in range(DC):
                        nc.tensor.matmul(po[:],
                                         aT[:, c, s * P:(s + 1) * P],
                                         wo_sb[l][:, c, :],
                                         start=(c == 0), stop=(c == DC - 1))
                    if flags['bo']:
                        nc.vector.tensor_add(po[:], po[:], vsl('bo', l, D))
                    ob = sp1.tile([P, D], BF16, tag="ob")
                    nc.scalar.copy(ob[:], po[:])
                    nc.vector.tensor_add(x[:, s, :], x[:, s, :], ob[:])

                _layernorm(nc, sp, sp1, x,
                           vsl('ln1_g', l, D) if flags['ln_g'] else None,
                           vsl('ln1_b', l, D) if flags['ln_b'] else None,
                           eps_sb[:, 0:1])

                # x1^T for FFN1
                x1T = st['x1T'] = b1p.tile([P, DC, S * P], BF16, tag="x1T", name="x1T")
                for s in range(S):
                    for c in range(DC):
                        nc.sync.dma_start_transpose(
                            x1T[:, c, s * P:(s + 1) * P],
                            x[:, s, c * P:(c + 1) * P])

            def stage_ffn_ln2(i, l):
                st = state[i]
                x, x1T = st['x'], st['x1T']
                # FFN1 -> h^T [dff-part, tok], fused relu (+b1)
                hT = b1p.tile([P, FCH, S * P], BF16, tag="hT")
                for m in range(FCH):
                    for hf in range(2):
                        ph = psB.tile([P, D], F32, tag="mm")
                        for c in range(DC):
                            nc.tensor.matmul(
                                ph[:],
                                w1_sb[l][:, c, m * P:(m + 1) * P],
                                x1T[:, c, hf * D:(hf + 1) * D],
                                start=(c == 0), stop=(c == DC - 1))
                        if flags['b1']:
                            nc.scalar.activation(
                                hT[:, m, hf * D:(hf + 1) * D], ph[:],
                                AF.Relu,
                                bias=b1t_sb[:, l * FCH + m:l * FCH + m + 1])
                        else:
                            nc.scalar.activation(
                                hT[:, m, hf * D:(hf + 1) * D], ph[:],
                                AF.Relu)

                # FFN2 + residual + LN2
                for s in range(S):
                    pf = psB.tile([P, D], F32, tag="mm")
                    for k in range(FCH):
                        nc.tensor.matmul(pf[:],
                                         hT[:, k, s * P:(s + 1) * P],
                                         w2_sb[l][:, k, :],
                                         start=(k == 0),
                                         stop=(k == FCH - 1))
                    if flags['b2']:
                        nc.vector.tensor_add(pf[:], pf[:], vsl('b2', l, D))
                    fb = sp1.tile([P, D], BF16, tag="fb")
                    nc.scalar.copy(fb[:], pf[:])
                    nc.vector.tensor_add(x[:, s, :], x[:, s, :], fb[:])
                _layernorm(nc, sp, sp1, x,
                           vsl('ln2_g', l, D) if flags['ln_g'] else None,
                           vsl('ln2_b', l, D) if flags['ln_b'] else None,
                           eps_sb[:, 0:1])

            def stage_cls(i):
                st = state[i]
                x = st['x']
                # max-pool over s + classifier
                nc.vector.tensor_tensor(out=x[:, 0:4, :], in0=x[:, 0:4, :],
                                        in1=x[:, 4:8, :], op=OP.max)
                nc.vector.tensor_tensor(out=x[:, 0:2, :], in0=x[:, 0:2, :],
                                        in1=x[:, 2:4, :], op=OP.max)
                nc.vector.tensor_tensor(out=x[:, 0, :], in0=x[:, 0, :],
                                        in1=x[:, 1, :], op=OP.max)
                rT = sp1.tile([P, DC, P], BF16, tag="rT")
                for c in range(DC):
                    nc.sync.dma_start_transpose(rT[:, c, :],
                                                x[:, 0, c * P:(c + 1) * P])
                pc = psB.tile([P, D], F32, tag="mm")
                for c in range(DC):
                    nc.tensor.matmul(pc[:, 0:NCLS], rT[:, c, :], wf_sb[:, c, :],
                                     start=(c == 0), stop=(c == DC - 1))
                if flags['bfc']:
                    nc.vector.tensor_add(pc[:, 0:NCLS], pc[:, 0:NCLS],
                                         vec_sb['bfc'][:, :])
                lg = sp1.tile([P, NCLS], F32, tag="lg")
                nc.vector.tensor_copy(lg[:], pc[:, 0:NCLS])
                nc.sync.dma_start(out_d[ds(i * P, P), :], lg[:])

            stages = [stage_gather]
            for l in range(NL):
                stages += [
                    (lambda i, l=l: stage_qkv(i, l)),
                    (lambda i, l=l: stage_attn(i, l)),
                    (lambda i, l=l: stage_wo_ln1(i, l)),
                    (lambda i, l=l: stage_ffn_ln2(i, l)),
                ]
            stages.append(stage_cls)
            NSTG = len(stages)

            # Skewed emission: tile i runs stage (step - SKEW*i). Older
            # tiles (deeper stages) are emitted first within a step so the
            # critical path leads each engine's in-order queue.
            # (For_i + in-loop SWDGE emits InstIncSwdgeSem, which this
            # container's walrus cannot encode -> fully unrolled anyway.)
            for step in range(SKEW * (NT - 1) + NSTG):
                for i in range(NT):
                    k = step - SKEW * i
                    if 0 <= k < NSTG:
                        stages[k](i)

    _split_multiwait_drains(nc)
    return nc


OPT_KEYS = ('bqkv', 'bo', 'b1', 'b2', 'bfc', 'ln_g', 'ln_b')
_cache = {}


def _get_nc(flags):
    key = tuple(flags[k] for k in OPT_KEYS)
    if key not in _cache:
        _cache[key] = build(flags)
    return _cache[key]


def _perm_eh(a):
    """Reorder the last axis (512 = 8 heads x 64 dims, h-major) to (e, h)
    h-minor order: new[..., e*8+h] = old[..., h*64+e]."""
    shp = a.shape
    return np.ascontiguousarray(
        a.reshape(*shp[:-1], H, DH).swapaxes(-2, -1).reshape(*shp))


def kernel(**inputs):
    token_ids = np.asarray(inputs['token_ids'])
    edge_src = np.asarray(inputs['edge_src'])
    emb = np.asarray(inputs['emb'], dtype=np.float32)
    Wqkv = np.asarray(inputs['Wqkv'], dtype=np.float32)
    bqkv = np.asarray(inputs['bqkv'], dtype=np.float32)
    Wo = np.asarray(inputs['Wo'], dtype=np.float32)
    bo = np.asarray(inputs['bo'], dtype=np.float32)
    W1 = np.asarray(inputs['W1'], dtype=np.float32)
    b1 = np.asarray(inputs['b1'], dtype=np.float32)
    W2 = np.asarray(inputs['W2'], dtype=np.float32)
    b2 = np.asarray(inputs['b2'], dtype=np.float32)
    ln1_g = np.asarray(inputs['ln1_g'], dtype=np.float32)
    ln1_b = np.asarray(inputs['ln1_b'], dtype=np.float32)
    ln2_g = np.asarray(inputs['ln2_g'], dtype=np.float32)
    ln2_b = np.asarray(inputs['ln2_b'], dtype=np.float32)
    Wfc = np.asarray(inputs['Wfc'], dtype=np.float32)
    bfc = np.asarray(inputs['bfc'], dtype=np.float32)

    flags = {
        'bqkv': bool(np.any(bqkv)), 'bo': bool(np.any(bo)),
        'b1': bool(np.any(b1)), 'b2': bool(np.any(b2)),
        'bfc': bool(np.any(bfc)),
        'ln_g': bool(np.any(ln1_g != 1.0) or np.any(ln2_g != 1.0)),
        'ln_b': bool(np.any(ln1_b) or np.any(ln2_b)),
    }
    nc = _get_nc(flags)

    bf = ml_dtypes.bfloat16
    tid2 = token_ids[edge_src[:, :S]].astype(np.int32)     # [NDST, S]
    embb = emb.astype(bf)
    # wqkvT: [NL, d_in, 3D] with q/k/v output columns permuted to (e, h)
    # order; q columns additionally pre-scaled by 1/sqrt(dh) = 1/8.
    wqkvT = np.ascontiguousarray(Wqkv.transpose(0, 2, 1))
    wq_c = _perm_eh(wqkvT[:, :, 0:D]) * 0.125
    wk_c = _perm_eh(wqkvT[:, :, D:2 * D])
    wv_c = _perm_eh(wqkvT[:, :, 2 * D:3 * D])
    wqkvT = np.concatenate([wq_c, wk_c, wv_c], axis=2).astype(bf)
    # woT: [NL, d_in, d_out] with input rows permuted to (e, h) to match
    # the attention-output column order.
    woT = np.ascontiguousarray(Wo.transpose(0, 2, 1))
    woT = np.ascontiguousarray(
        woT.reshape(NL, H, DH, D).swapaxes(1, 2).reshape(NL, D, D)).astype(bf)
    w1T = np.ascontiguousarray(W1.transpose(0, 2, 1)).astype(bf)
    w2T = np.ascontiguousarray(W2.transpose(0, 2, 1)).astype(bf)
    wfcT = np.ascontiguousarray(Wfc.T).astype(bf)

    common = {
        'embb': embb, 'wqkvT': wqkvT, 'woT': woT, 'w1T': w1T, 'w2T': w2T,
        'wfcT': wfcT,
    }
    if flags['bqkv']:
        bq_c = _perm_eh(bqkv[:, 0:D]) * 0.125
        bk_c = _perm_eh(bqkv[:, D:2 * D])
        bv_c = _perm_eh(bqkv[:, 2 * D:3 * D])
        common['bqkv'] = np.concatenate([bq_c, bk_c, bv_c], axis=1)
    if flags['bo']:
        common['bo'] = bo
    if flags['b1']:
        # [P, NL*FCH]: partition p, col l*FCH+m = b1[l, m*P+p]
        common['b1t'] = np.ascontiguousarray(
            b1.reshape(NL, FCH, P).transpose(2, 0, 1).reshape(P, NL * FCH))
    if flags['b2']:
        common['b2'] = b2
    if flags['bfc']:
        common['bfc'] = bfc.reshape(1, NCLS)
    if flags['ln_g']:
        common['ln1_g'] = ln1_g
        common['ln2_g'] = ln2_g
    if flags['ln_b']:
        common['ln1_b'] = ln1_b
        common['ln2_b'] = ln2_b

    in_maps = []
    for c in range(NCORES):
        m = dict(common)
        m['tid2'] = np.ascontiguousarray(tid2[c * NLOC:(c + 1) * NLOC])
        in_maps.append(m)

    res = run_bass_kernel_spmd(nc, in_maps, core_ids=list(range(NCORES)))
    out = np.concatenate([res.results[c]['logits'] for c in range(NCORES)],
                         axis=0)
    return out.astype(np.float32)


if __name__ == '__main__':
    import time
    sys.path.insert(0, '/root/problem')
    import reference
    inp = {k: np.asarray(v) for k, v in reference.setup_inputs().items()}
    t0 = time.time()
    got = kernel(**inp)
    print(f"kernel ran in {time.time()-t0:.1f}s")
    exp = np.asarray(reference.reference(**reference.setup_inputs()))
    err = np.abs(got - exp).max()
    rel = err / np.abs(exp).max()
    print(f"absmax err {err:.3e}  rel {rel:.3e}")
